# revision 2
# baseline (speedup 1.0000x reference)
"""Trainium2 Bass kernel: GNN message passing (metasurface inverse design).

Distribution (8 NeuronCores), v2:
  - Edges sharded by TARGET-node range (12500 nodes per core).
  - Node state h replicated each layer via AllGather (bf16).
  - Per-edge gather h[src] via SWDGE dma_gather (cheap: ~1.6ns/desc).
  - Aggregation by target = PE matmuls against host-built one-hot
    "selection" matrices S (bf16), accumulated in PSUM per 4-block
    supergroup.  This replaces the baseline's dma_scatter_add, whose
    HBM read-modify-write ran at ~7ns/desc and dominated the runtime.
  - All per-node dense work is feature-major ([128 feat partitions x
    node columns]), which turns biases into per-partition scalars and
    needs no transposes inside the layer loop.

Math (linearity of the edge matmul over the segment-sum):
    agg = G @ W1 + deg * (h @ W2) + deg x bm,   G = segsum_tgt(h[src])
  with Wm[l] = [W1; W2] and deg = in-degree.

Uniform-program constraint: all 8 cores run ONE compiled program, so
slot/chunk/piece structure is computed from per-(sg,window) maxima over
cores; only tensor contents (gidx, S) differ per core.
"""

import os

import numpy as np
import ml_dtypes

import concourse.bacc as bacc
import concourse.mybir as mybir
import concourse.tile as tile
from concourse.bass_utils import run_bass_kernel_spmd
from concourse.masks import make_identity

F32 = mybir.dt.float32
BF16 = mybir.dt.bfloat16
I16 = mybir.dt.int16
NCORES = 8
H = 128
P = 128
TWO_PI = 6.283185307179586
BF = ml_dtypes.bfloat16


class Cfg:
    def __init__(self, N, E, B, M, L):
        self.N, self.E, self.B, self.M, self.L = N, E, B, M, L
        self.ns = N // NCORES            # 12500 nodes per core
        self.nsp = 12800                 # padded (100 blocks of 128)
        self.nbl = self.nsp // 128       # 100 local blocks
        self.nsg = self.nbl // 4         # 25 supergroups (1 psum bank each)
        self.win = 25600                 # gather idx window (int16 range)
        self.npad = NCORES * self.nsp    # 102400 padded h_full rows
        self.nw = self.npad // self.win  # 4 windows


def _pack(cfg, src, tgt):
    """Build uniform slot/chunk/piece structure + per-core gidx and S."""
    ns, nsg, nw = cfg.ns, cfg.nsg, cfg.nw
    core = tgt // ns
    tl = tgt - core * ns                 # local tgt 0..12499
    blk = tl // 128                      # local block 0..97
    col = tl - blk * 128
    sg = blk // 4
    srow = (src // ns) * cfg.nsp + (src % ns)   # row of src in h_full
    w = srow // cfg.win
    wl = (srow - w * cfg.win).astype(np.int64)

    key = (core * nsg + sg) * nw + w
    order = np.lexsort((src, tl, key))
    cnt = np.bincount(key, minlength=NCORES * nsg * nw).reshape(NCORES, nsg, nw)
    run_len = ((cnt.max(axis=0) + 127) // 128) * 128       # [nsg, nw]
    flat = run_len.reshape(-1)
    run_off = np.concatenate([[0], np.cumsum(flat)[:-1]]).reshape(nsg, nw)
    SLOTS = int(flat.sum())
    NCH = SLOTS // 128

    st = np.concatenate([[0], np.cumsum(cnt.reshape(-1))[:-1]])
    pos = np.empty(len(src), np.int64)
    pos[order] = np.arange(len(src)) - st[key[order]]
    slot = run_off[sg, w] + pos

    gidx = np.zeros((NCORES, SLOTS), np.int16)
    gidx[core, slot] = wl.astype(np.int16)
    gidx_w = np.ascontiguousarray(
        gidx.reshape(NCORES, SLOTS // 16, 16).transpose(0, 2, 1)
    )
    gidx_in = np.tile(gidx_w, (1, 8, 1))                   # [8, 128, SLOTS/16]

    # chunk -> sg map and per-sg chunk ranges
    ch_per_sg = (run_len // 128).sum(axis=1)               # [nsg]
    sg_ch0 = np.concatenate([[0], np.cumsum(ch_per_sg)])   # [nsg+1]

    # pieces: union over cores of blocks present in each chunk
    bval = np.full((NCORES, SLOTS), -1, np.int64)
    bval[core, slot] = blk
    bs = bval.reshape(NCORES, NCH, 128)
    piece_list = []                                        # (chunk, block)
    have = set()
    for ch in range(NCH):
        blocks = np.unique(bs[:, ch, :])
        for b in blocks[blocks >= 0]:
            piece_list.append((ch, int(b)))
            have.add(int(b))
    for b in range(cfg.nbl):                               # empty pad blocks
        if b not in have:
            piece_list.append((int(sg_ch0[b // 4]), b))
    piece_list.sort()
    NP = len(piece_list)
    pid = {pb: i for i, pb in enumerate(piece_list)}

    # start/stop flags per block (in emission order)
    first = {}
    last = {}
    for i, (ch, b) in enumerate(piece_list):
        if b not in first:
            first[b] = i
        last[b] = i

    # per-sg piece ranges
    sg_of_piece = np.array([b // 4 for _, b in piece_list])
    sg_p0 = np.searchsorted(sg_of_piece, np.arange(nsg + 1))

    # per-core S (bf16): one 1 per edge at (piece, slot%128, col)
    pidx_map = np.full((NCH, cfg.nbl), -1, np.int64)
    for i, (ch, b) in enumerate(piece_list):
        pidx_map[ch, b] = i
    S_all = []
    for c in range(NCORES):
        m = core == c
        pc = pidx_map[(slot[m] // 128), blk[m]]
        assert (pc >= 0).all()
        S = np.zeros((NP, 128, 128), BF)
        S[pc, slot[m] % 128, col[m]] = 1.0
        S_all.append(S.reshape(NP * 128, 128))

    meta = {
        "SLOTS": SLOTS,
        "NCH": NCH,
        "NP": NP,
        "run_len": run_len,
        "run_off": run_off,
        "sg_ch0": sg_ch0,
        "sg_p0": sg_p0,
        "pieces": piece_list,
        "first": first,
        "last": last,
    }
    return gidx_in, S_all, meta


def _build(cfg, meta):
    phase = os.environ.get("GNN_PHASE", "full")
    NS, NSP, B, M, L = cfg.ns, cfg.nsp, cfg.B, cfg.M, cfg.L
    NSG, NW, WIN = cfg.nsg, cfg.nw, cfg.win
    NT = NSP // 512                      # 25 dense 512-col tiles
    MT = M // 128
    SLOTS, NCH, NP = meta["SLOTS"], meta["NCH"], meta["NP"]
    run_len, run_off = meta["run_len"], meta["run_off"]
    sg_ch0, sg_p0 = meta["sg_ch0"], meta["sg_p0"]
    pieces, first, last = meta["pieces"], meta["first"], meta["last"]
    EVCH = int(max(sg_ch0[s + 1] - sg_ch0[s] for s in range(NSG)))
    SPC = int(max(sg_p0[s + 1] - sg_p0[s] for s in range(NSG)))

    nc = bacc.Bacc(None, num_devices=NCORES)

    x_s = nc.dram_tensor("x_s", [NSP, H], F32, kind="ExternalInput")
    Wemb = nc.dram_tensor("Wemb", [H, H], F32, kind="ExternalInput")
    bemb = nc.dram_tensor("bemb", [H, 1], F32, kind="ExternalInput")
    Wm = nc.dram_tensor("Wm", [L, 2 * H, H], F32, kind="ExternalInput")
    bmr = nc.dram_tensor("bmr", [L, 1, H], F32, kind="ExternalInput")
    degrow_d = nc.dram_tensor("degrow_d", [1, NSP], F32, kind="ExternalInput")
    W1r = nc.dram_tensor("W1r", [H, 2 * H], F32, kind="ExternalInput")
    b1r = nc.dram_tensor("b1r", [H, 2], F32, kind="ExternalInput")
    W2r = nc.dram_tensor("W2r", [2 * H, M], F32, kind="ExternalInput")
    b2r = nc.dram_tensor("b2r", [H, MT], F32, kind="ExternalInput")
    gidx = nc.dram_tensor("gidx", [P, SLOTS // 16], I16, kind="ExternalInput")
    S_d = nc.dram_tensor("S_d", [NP * 128, 128], BF16, kind="ExternalInput")
    degb_d = nc.dram_tensor("degb_d", [P, NSP], BF16, kind="ExternalInput")
    ind = nc.dram_tensor("ind", [P, (NSP // 128) * B], F32, kind="ExternalInput")

    outT = nc.dram_tensor("outT", [M, B], F32, kind="ExternalOutput")
    dbg = os.environ.get("GNN_DEBUG")
    if dbg:
        dbg_ev = nc.dram_tensor("dbg_ev", [P, EVCH * H], BF16, kind="ExternalOutput")
        dbg_agg = nc.dram_tensor("dbg_agg", [P, NSP], F32, kind="ExternalOutput")
        dbg_h = nc.dram_tensor("dbg_h", [NSP, H], BF16, kind="ExternalOutput")

    h_sh = nc.dram_tensor("h_sh", [NSP, H], BF16)
    h_full = nc.dram_tensor("h_full", [cfg.npad, H], BF16, addr_space="Shared")
    agg_d = nc.dram_tensor("agg_d", [P, NSP], F32)
    gpart = nc.dram_tensor("gpart", [H, B], F32)
    gsum = nc.dram_tensor("gsum", [H, B], F32, addr_space="Shared")

    groups = [list(range(NCORES))]

    with tile.TileContext(nc) as tc:
        with (
            tc.tile_pool(name="const", bufs=1) as cp,
            tc.tile_pool(name="work", bufs=2) as wp,
            tc.tile_pool(name="stage", bufs=3) as stp,
        ):
            pp_cm = tc.tile_pool(name="psum", bufs=2, space="PSUM")
            pp = pp_cm.__enter__()
            ep_cm = tc.tile_pool(name="edge", bufs=2)
            ep = ep_cm.__enter__()
            sp_cm = tc.tile_pool(name="smat", bufs=2)
            sp = sp_cm.__enter__()

            ident = cp.tile([P, P], F32, tag="ident")
            make_identity(nc, ident[:])
            Wemb_sb = cp.tile([H, H], F32, tag="wemb")
            nc.sync.dma_start(Wemb_sb[:], Wemb[:])
            bemb_sb = cp.tile([H, 1], F32, tag="bemb")
            nc.sync.dma_start(bemb_sb[:], bemb[:])
            W1_sb, W2_sb, bm_sb = [], [], []
            for l in range(L):
                w1 = cp.tile([H, H], F32, tag=f"w1_{l}")
                nc.sync.dma_start(w1[:], Wm[l, :H, :])
                w2 = cp.tile([H, H], F32, tag=f"w2_{l}")
                nc.sync.dma_start(w2[:], Wm[l, H:, :])
                bm_t = cp.tile([1, H], F32, tag=f"bm_{l}")
                nc.sync.dma_start(bm_t[:], bmr[l, :, :])
                W1_sb.append(w1)
                W2_sb.append(w2)
                bm_sb.append(bm_t)
            degb = cp.tile([P, NSP], BF16, tag="degb")
            nc.sync.dma_start(degb[:], degb_d[:])
            h_fm = cp.tile([P, NSP], F32, tag="h_fm")

            # ---- embed: h0 = x @ Wemb + bemb, kept feature-major ----
            for t in range(NT):
                rows = slice(512 * t, 512 * (t + 1))
                xt = wp.tile([P, 4 * H], F32, tag="in_a")
                nc.sync.dma_start(
                    xt[:].rearrange("p (b f) -> p b f", f=H),
                    x_s[rows, :].rearrange("(b p) f -> p b f", p=P),
                )
                ps_x = pp.tile([P, 512], F32, tag="ps_t")
                for b in range(4):
                    nc.tensor.transpose(
                        ps_x[:, b * H : (b + 1) * H],
                        xt[:, b * H : (b + 1) * H],
                        ident[:],
                    )
                xT = wp.tile([P, 512], F32, tag="t_a")
                nc.vector.tensor_copy(xT[:], ps_x[:])
                ps_h = pp.tile([P, 512], F32, tag="ps2")
                nc.tensor.matmul(ps_h[:], Wemb_sb[:], xT[:], start=True, stop=True)
                nc.vector.tensor_scalar_add(
                    h_fm[:, rows], ps_h[:], bemb_sb[:, 0:1]
                )
                # node-major bf16 copy for AllGather
                ps_nm = pp.tile([P, 512], F32, tag="ps_t")
                for b in range(4):
                    nc.tensor.transpose(
                        ps_nm[:, b * H : (b + 1) * H],
                        h_fm[:, 512 * t + b * H : 512 * t + (b + 1) * H],
                        ident[:],
                    )
                hn = wp.tile([P, 4 * H], BF16, tag="out_a")
                nc.vector.tensor_copy(hn[:], ps_nm[:])
                nc.sync.dma_start(
                    h_sh[rows, :].rearrange("(b p) f -> p b f", p=P),
                    hn[:].rearrange("p (b f) -> p b f", f=H),
                )

            nc.gpsimd.collective_compute(
                "AllGather",
                mybir.AluOpType.bypass,
                replica_groups=groups,
                ins=[h_sh[:, :]],
                outs=[h_full[:, :]],
            )

            # ---- message-passing layers ----
            for l in range(L):
                # stage 1: gather + S-matmul aggregation per supergroup
                for s in range(NSG if phase != "noedge" else 0):
                    c0, c1 = int(sg_ch0[s]), int(sg_ch0[s + 1])
                    nch = c1 - c0
                    slots_sg = nch * 128
                    idx_t = ep.tile([P, (EVCH * 128) // 16], I16, tag="idx")
                    nc.sync.dma_start(
                        idx_t[:, : slots_sg // 16],
                        gidx[:, (c0 * 128) // 16 : (c1 * 128) // 16],
                    )
                    ev = ep.tile([P, EVCH * H], BF16, tag="ev")
                    evv = ev[:].rearrange("p (c e) -> p c e", e=H)
                    for w in range(NW):
                        n = int(run_len[s, w])
                        if n == 0:
                            continue
                        r0 = (int(run_off[s, w]) - c0 * 128) // 128
                        nc.gpsimd.dma_gather(
                            evv[:, r0 : r0 + n // 128, :],
                            h_full[w * WIN : (w + 1) * WIN, :],
                            idx_t[:, (r0 * 128) // 16 : (r0 * 128 + n) // 16],
                            n,
                            n,
                            H,
                            single_packet=False,
                        )
                    p0, p1 = int(sg_p0[s]), int(sg_p0[s + 1])
                    S_t = sp.tile([P, SPC * 128], BF16, tag="s")
                    nc.sync.dma_start(
                        S_t[:, : (p1 - p0) * 128].rearrange(
                            "p (c t) -> p c t", t=128
                        ),
                        S_d[p0 * 128 : p1 * 128, :].rearrange(
                            "(c p) t -> p c t", p=P
                        ),
                    )
                    bank = pp.tile([P, 512], F32, tag="bank", name=f"bank_{l}_{s}")
                    for i in range(p0, p1):
                        ch, b = pieces[i]
                        q = b % 4
                        j = i - p0
                        cl = ch - c0
                        # start=True zeroes the ENTIRE psum bank (not just the
                        # addressed region) -> exactly one start/stop per bank.
                        nc.tensor.matmul(
                            bank[:, q * 128 : (q + 1) * 128],
                            ev[:, cl * H : (cl + 1) * H],
                            S_t[:, j * 128 : (j + 1) * 128],
                            start=(i == p0),
                            stop=(i == p1 - 1),
                        )
                    stg = stp.tile([P, 512], F32, tag="stg")
                    nc.vector.tensor_copy(stg[:], bank[:])
                    nc.sync.dma_start(agg_d[:, s * 512 : (s + 1) * 512], stg[:])
                    if dbg and l == 0 and s == 0:
                        nc.sync.dma_start(dbg_ev[:, :], ev[:, :])

                # stage 2: h = relu(h + G@W1 + deg*(h@W2) + deg x bm)
                for t in range(NT if phase != "nodense" else 0):
                    cols = slice(512 * t, 512 * (t + 1))
                    Gt = wp.tile([P, 512], F32, tag="gt")
                    nc.sync.dma_start(Gt[:], agg_d[:, cols])
                    dgt = wp.tile([P, 512], F32, tag="dgt")
                    nc.vector.tensor_copy(dgt[:], degb[:, cols])
                    Hd = wp.tile([P, 512], F32, tag="hd")
                    nc.vector.tensor_mul(Hd[:], h_fm[:, cols], dgt[:])
                    drw = wp.tile([1, 512], F32, tag="drw")
                    nc.sync.dma_start(drw[:], degrow_d[:, cols])
                    ps2 = pp.tile([P, 512], F32, tag="ps2")
                    nc.tensor.matmul(ps2[:], W1_sb[l][:], Gt[:], start=True, stop=False)
                    nc.tensor.matmul(ps2[:], W2_sb[l][:], Hd[:], start=False, stop=False)
                    nc.tensor.matmul(ps2[:], bm_sb[l][:], drw[:], start=False, stop=True)
                    t1 = wp.tile([P, 512], F32, tag="t1")
                    nc.vector.tensor_add(t1[:], ps2[:], h_fm[:, cols])
                    nc.vector.tensor_scalar_max(h_fm[:, cols], t1[:], 0.0)
                    if l < L - 1:
                        ps_nm = pp.tile([P, 512], F32, tag="ps_t")
                        for b in range(4):
                            nc.tensor.transpose(
                                ps_nm[:, b * H : (b + 1) * H],
                                h_fm[:, 512 * t + b * H : 512 * t + (b + 1) * H],
                                ident[:],
                            )
                        hn = wp.tile([P, 4 * H], BF16, tag="out_a")
                        nc.vector.tensor_copy(hn[:], ps_nm[:])
                        rows = slice(512 * t, 512 * (t + 1))
                        nc.sync.dma_start(
                            h_sh[rows, :].rearrange("(b p) f -> p b f", p=P),
                            hn[:].rearrange("p (b f) -> p b f", f=H),
                        )

                if dbg and l == 0:
                    nc.sync.dma_start(dbg_agg[:, :], agg_d[:, :])
                    nc.sync.dma_start(dbg_h[:, :], h_sh[:, :])
                if l < L - 1:
                    nc.gpsimd.collective_compute(
                        "AllGather",
                        mybir.AluOpType.bypass,
                        replica_groups=groups,
                        ins=[h_sh[:, :]],
                        outs=[h_full[:, :]],
                    )

            # ---- readout: g = per-graph mean (ind holds 1/count) ----
            sp_cm.__exit__(None, None, None)
            ep_cm.__exit__(None, None, None)
            pp_cm.__exit__(None, None, None)
            pp_cm = tc.tile_pool(name="psum_ro", bufs=1, space="PSUM")
            pp = pp_cm.__enter__()
            rp_cm = tc.tile_pool(name="ro", bufs=1)
            rp = rp_cm.__enter__()
            ind_sb = rp.tile([P, (NSP // 128) * B], F32, tag="ind")
            nc.sync.dma_start(ind_sb[:], ind[:])
            ps_gr = pp.tile([P, B], F32, tag="ps_gr")
            NBLK = NSP // 128
            for t in range(NT):
                ps_ro = pp.tile([P, 512], F32, tag="ps_t")
                for b in range(4):
                    nc.tensor.transpose(
                        ps_ro[:, b * H : (b + 1) * H],
                        h_fm[:, 512 * t + b * H : 512 * t + (b + 1) * H],
                        ident[:],
                    )
                hro = wp.tile([P, 512], F32, tag="hro")
                nc.vector.tensor_copy(hro[:], ps_ro[:])
                for b in range(4):
                    blk = 4 * t + b
                    nc.tensor.matmul(
                        ps_gr[:],
                        hro[:, b * H : (b + 1) * H],
                        ind_sb[:, blk * B : (blk + 1) * B],
                        start=(blk == 0),
                        stop=(blk == NBLK - 1),
                    )
            gp_sb = wp.tile([P, B], F32, tag="gp")
            nc.vector.tensor_copy(gp_sb[:], ps_gr[:])
            nc.sync.dma_start(gpart[:, :], gp_sb[:])
            nc.gpsimd.collective_compute(
                "AllReduce",
                mybir.AluOpType.add,
                replica_groups=groups,
                ins=[gpart[:, :]],
                outs=[gsum[:, :]],
            )
            gs_sb = wp.tile([P, B], F32, tag="gs")
            nc.sync.dma_start(gs_sb[:], gsum[:, :])

            W1r_sb = rp.tile([H, 2 * H], F32, tag="w1r")
            nc.sync.dma_start(W1r_sb[:], W1r[:])
            b1_sb = rp.tile([H, 2], F32, tag="b1r")
            nc.sync.dma_start(b1_sb[:], b1r[:])
            W2ra_sb = rp.tile([H, M], F32, tag="w2ra")
            nc.sync.dma_start(W2ra_sb[:], W2r[0:H, :])
            W2rb_sb = rp.tile([H, M], F32, tag="w2rb")
            nc.sync.dma_start(W2rb_sb[:], W2r[H:, :])
            b2_sb = rp.tile([H, MT], F32, tag="b2r")
            nc.sync.dma_start(b2_sb[:], b2r[:])

            z1 = []
            for i in range(2):
                ps_z = pp.tile([P, B], F32, tag=f"ps_z{i}")
                nc.tensor.matmul(
                    ps_z[:], W1r_sb[:, i * H : (i + 1) * H], gs_sb[:],
                    start=True, stop=True,
                )
                zt = wp.tile([P, B], F32, tag=f"z1_{i}")
                nc.vector.tensor_scalar(
                    zt[:], ps_z[:], b1_sb[:, i : i + 1], 0.0,
                    mybir.AluOpType.add, mybir.AluOpType.max,
                )
                z1.append(zt)

            o_sb = wp.tile([P, MT * B], F32, tag="o")
            for m in range(MT):
                ps_o = pp.tile([P, B], F32, tag="ps_o")
                ms = slice(m * H, (m + 1) * H)
                nc.tensor.matmul(ps_o[:], W2ra_sb[:, ms], z1[0][:], start=True, stop=False)
                nc.tensor.matmul(ps_o[:], W2rb_sb[:, ms], z1[1][:], start=False, stop=True)
                nc.scalar.activation(
                    o_sb[:, m * B : (m + 1) * B],
                    ps_o[:],
                    mybir.ActivationFunctionType.Sigmoid,
                    bias=b2_sb[:, m : m + 1],
                    scale=1.0,
                )
            nc.vector.tensor_scalar_mul(o_sb[:], o_sb[:], TWO_PI)
            nc.sync.dma_start(
                outT[:, :].rearrange("(m p) b -> p m b", p=P),
                o_sb[:].rearrange("p (m b) -> p m b", b=B),
            )
            rp_cm.__exit__(None, None, None)
            pp_cm.__exit__(None, None, None)

    nc.finalize()
    return nc


def _run(inputs, cfg, trace=False):
    x = np.asarray(inputs["x"], np.float32)
    ei = np.asarray(inputs["edge_index"])
    batch = np.asarray(inputs["batch"]).astype(np.int64)
    W_embed = np.asarray(inputs["W_embed"], np.float32)
    b_embed = np.asarray(inputs["b_embed"], np.float32)
    Wm = np.asarray(inputs["Wm"], np.float32)
    bm = np.asarray(inputs["bm"], np.float32)
    W1 = np.asarray(inputs["W1"], np.float32)
    b1 = np.asarray(inputs["b1"], np.float32)
    W2 = np.asarray(inputs["W2"], np.float32)
    b2 = np.asarray(inputs["b2"], np.float32)

    src = np.asarray(ei[0], np.int64)
    tgt = np.asarray(ei[1], np.int64)
    NS, NSP, B, M, L = cfg.ns, cfg.nsp, cfg.B, cfg.M, cfg.L

    gidx_in, S_all, meta = _pack(cfg, src, tgt)

    deg = np.bincount(tgt, minlength=cfg.N).astype(np.float32)
    counts = np.bincount(batch, minlength=B).astype(np.float32)
    invc = 1.0 / np.clip(counts, 1.0, None)

    in_maps = []
    for c in range(NCORES):
        sl = slice(c * NS, (c + 1) * NS)
        x_c = np.zeros((NSP, H), np.float32)
        x_c[:NS] = x[sl]
        deg_c = np.zeros(NSP, np.float32)
        deg_c[:NS] = deg[sl]
        degb_c = np.ascontiguousarray(
            np.broadcast_to(deg_c[None, :].astype(BF), (P, NSP))
        )
        ind_c = np.zeros((NSP, B), np.float32)
        ind_c[np.arange(NS), batch[sl]] = invc[batch[sl]]
        in_maps.append(
            {
                "x_s": x_c,
                "Wemb": W_embed,
                "bemb": b_embed.reshape(H, 1),
                "Wm": Wm,
                "bmr": np.ascontiguousarray(bm.reshape(L, 1, H)),
                "degrow_d": deg_c.reshape(1, NSP),
                "W1r": W1,
                "b1r": np.ascontiguousarray(b1.reshape(2, H).T),
                "W2r": W2,
                "b2r": np.ascontiguousarray(b2.reshape(M // 128, H).T),
                "gidx": gidx_in[c],
                "S_d": S_all[c],
                "degb_d": degb_c,
                "ind": np.ascontiguousarray(
                    ind_c.reshape(NSP // 128, P, B)
                    .transpose(1, 0, 2)
                    .reshape(P, (NSP // 128) * B)
                ),
            }
        )

    nc = _build(cfg, meta)
    res = run_bass_kernel_spmd(
        nc, in_maps, core_ids=list(range(NCORES)), trace=trace
    )
    out = np.ascontiguousarray(res.results[0]["outT"].T)
    return out, res


def kernel(**inputs) -> np.ndarray:
    cfg = Cfg(N=100000, E=1600000, B=16, M=2048, L=3)
    trace = bool(os.environ.get("GNN_TRACE"))
    out, _ = _run(inputs, cfg, trace=trace)
    return out
